# revision 8
# baseline (speedup 1.0000x reference)
"""MoE QLoRA linear kernel for Trainium2 (8 NeuronCores, data-parallel over tokens).

Computes, for x:(B,S,IN) f32:
    base  = x @ W.T + b
    gates = softmax(x @ Wr.T)                       # (tok, E)
    proj  = x @ A[e].T                              # (tok, E, R)
    out   = base + sum_e SCALE * gates[...,e] * (proj[...,e,:] @ Bm[e].T)

Key algebraic fold: the gated expert mix is a single rank-(E*R) matmul:
    wproj[t, er] = SCALE * gates[t, e] * proj[t, er]          (er = e*R+r)
    lora[t, o]   = sum_er wproj[t, er] * Bcat[er, o]          (Bcat[er,o] = Bm[e,o,r])
and the bias b is folded in as an extra contraction row (wproj row of ones,
Bcat row = b), so base+lora+bias all accumulate in one PSUM group on the PE.

Per-core kernel (1024 tokens), everything oriented (feature-partition, token-free):
  phase 1: PSUM(36,512) = [A;Wr]^T-stationary matmuls over 32 k-tiles ->
           proj rows 0..31, router logits rows 32..35; softmax via exp +
           PE ones-matmul partition reductions/broadcasts; wproj written fp16.
           The o-tile-0 base k-loop is emitted between the proj matmuls and
           the gating chain so the PE stays busy while ACT/DVE run softmax.
  phase 2: for each of 32 o-tiles: out(128o, t) = W-tile-stationary matmul
           over 32 k-tiles + one lora matmul (k=33) accumulated into PSUM,
           copy to SBUF, DMA out as (OUT, tok); host transposes back.

All matmul inputs are fp16 (host-cast; PE runs fp16 at full bf16 rate,
fp32 PSUM accumulation). Host pre-tiles all layouts so every DMA is
contiguous and the kernel needs zero on-chip transposes.

Perf note: this toolchain's walrus runs with --enable-ldw-opt=false (=true
crashes codegen), so every matmul gets its own LDWEIGHTS (~53ns each,
~115us/core) on top of the ~490us cost-model prediction; measured device
time is ~600-650us/core. Matmul count (2182) is at the hardware floor
(m<=128, n<=512/PSUM bank, k<=128), so no tiling change reduces it. The
timeline trace shows the PE sequencer saturated end-to-end; PE-engine idle
is only startup DMA (~11us, both alternate DMA rings measured worse) and
the framework tail drain (~5us).

Known unexploited optimization (identified, not landed): the post-Tile BIR
contains 1068 redundant consecutive InstLdweights (second load of each
same-stationary (o-tile, k) pair), all wait/update-free and hence deletable
for ~57us (~9%). Landing it requires deleting them from the compiled
module's PE instruction stream (walrus pairs a standalone InstLdweights
with following non-self-loading matmuls for 2-byte dtypes), then full
hardware re-validation of numerics on all 8 cores.
"""

import numpy as np

import concourse.bass as bass
import concourse.tile as tile
from concourse import bacc, mybir
from concourse import bass_utils

# Problem shape (hardcoded; kernel.py must be self-contained)
B, S, IN, OUT, E, R = 4, 2048, 4096, 4096, 4, 8
SCALE = 16.0 / 8.0
N_CORES = 8
TOK = B * S                  # 8192 tokens
TPC = TOK // N_CORES         # 1024 tokens per core
P = 128                      # partitions
KT = IN // P                 # 32 k-tiles (contraction)
OT = OUT // P                # 32 output tiles
NSLAB = 512                  # moving-operand free size (PSUM bank = 512 f32)
NS = TPC // NSLAB            # 2 token slabs per core
ER = E * R                   # 32 low-rank rows
ERA = ER + 1                 # +1 ones row (bias fold)

F16 = mybir.dt.float16
F32 = mybir.dt.float32

_NC = None

# Post-compile pass: delete redundant consecutive InstLdweights from the PE
# stream. Walrus (--enable-ldw-opt=false) emits one LDWEIGHTS per matmul;
# when consecutive matmuls share the same stationary tile (the two token
# slabs of each (o-tile, k)), the second load is identical, wait/update-free,
# and costs ~53ns of serialized PE time. Deleting it lets the following
# non-self-loading matmul reuse the already-loaded weights.
DEDUPE_LDW = True


def _dedupe_ldweights(nc):
    removed = 0
    for blk in nc.m.functions[0].blocks:
        keep = []
        prev_sig = None
        for inst in blk.instructions:
            if isinstance(inst, mybir.InstLdweights):
                sig = str(inst.ins[0])
                if sig == prev_sig and not inst.has_wait() and not inst.has_update():
                    removed += 1
                    continue
                prev_sig = sig
            elif isinstance(inst, mybir.InstMatmult) and inst.ldweights is not False:
                # self-loading matmul (fp32 gating) reloads the PE array's
                # stationary weights — following matmuls need a fresh LDW
                prev_sig = None
            keep.append(inst)
        blk.instructions = keep
    return removed


def build_nc(reps=1, ns=NS):
    NS_ = ns
    nc = bacc.Bacc("TRN2", target_bir_lowering=False, debug=False)

    xd = nc.dram_tensor("xd", [P, KT, TPC], F16, kind="ExternalInput")
    wd = nc.dram_tensor("wd", [OT, P, KT, P], F16, kind="ExternalInput")
    artd = nc.dram_tensor("artd", [P, KT, ER + E], F16, kind="ExternalInput")
    btd = nc.dram_tensor("btd", [ERA, OUT], F16, kind="ExternalInput")
    seld = nc.dram_tensor("seld", [E, ER], F32, kind="ExternalInput")
    od = nc.dram_tensor("od", [OUT, TPC], F32, kind="ExternalOutput")

    with tile.TileContext(nc) as tc:
        with (
            tc.tile_pool(name="consts", bufs=1) as consts,
            tc.tile_pool(name="wpool", bufs=3) as wpool,
            tc.tile_pool(name="opool", bufs=3) as opool,
            tc.tile_pool(name="small", bufs=2) as small,
            tc.tile_pool(name="psum_proj", bufs=1, space="PSUM") as psum_proj,
            tc.tile_pool(name="psum_base", bufs=2, space="PSUM") as psum_base,
        ):
            # DMA issue order tracks the PE consumption schedule: phase 1
            # consumes art k-tiles + x k-tiles in order from t=0; W tile 0
            # isn't needed until the o-tile-0 k-loop (~15us in), W tile 1 and
            # the bias/sel tiles later still. Keeping the big W loads out of
            # the head of the queue cuts the PE's startup DMA wait.
            art_sb = consts.tile([P, KT, ER + E], F16)
            nc.sync.dma_start(out=art_sb[:, 0:4, :], in_=artd[:, 0:4, :])

            w_tiles = {}

            def load_w(ot):
                w_sb = wpool.tile([P, KT, P], F16, tag="w", name="w_sb")
                nc.sync.dma_start(out=w_sb[:], in_=wd[ot])
                w_tiles[ot] = w_sb

            # Resident activations: x^T tiled (p=i%128, k=i//128, t), fp16, 8 MiB.
            x_sb = consts.tile([P, KT, TPC], F16)
            nc.sync.dma_start(out=x_sb[:, 0, :], in_=xd[:, 0, :])
            nc.sync.dma_start(out=art_sb[:, 4:, :], in_=artd[:, 4:, :])
            for k in range(1, 4):
                nc.sync.dma_start(out=x_sb[:, k, :], in_=xd[:, k, :])
            sel_sb = consts.tile([E, ER], F32)
            nc.sync.dma_start(out=sel_sb[:], in_=seld[:])
            for k in range(4, 7):
                nc.sync.dma_start(out=x_sb[:, k, :], in_=xd[:, k, :])
            load_w(0)
            for k in range(7, 10):
                nc.sync.dma_start(out=x_sb[:, k, :], in_=xd[:, k, :])
            bt_sb = consts.tile([ERA, OUT], F16)
            nc.sync.dma_start(out=bt_sb[:], in_=btd[:])
            for k in range(10, 13):
                nc.sync.dma_start(out=x_sb[:, k, :], in_=xd[:, k, :])
            load_w(1)
            for k in range(13, KT):
                nc.sync.dma_start(out=x_sb[:, k, :], in_=xd[:, k, :])

            ones_e1 = consts.tile([E, 1], F32)
            nc.vector.memset(ones_e1[:], 1.0)
            ones_1e = consts.tile([1, E], F32)
            nc.vector.memset(ones_1e[:], 1.0)
            # Gated low-rank projection, fp16, rows 0..31 = wproj, row 32 = ones.
            wp_sb = consts.tile([ERA, TPC], F16)
            nc.vector.memset(wp_sb[ER : ER + 1, :], 1.0)

            # ---------- phase 1: proj + router matmul PSUM tiles ----------
            # rows 0..31: proj^T (er, t); rows 32..35: router logits (e, t)
            pps = [
                psum_proj.tile([ER + E, NSLAB], F32, tag=f"pp{t}", name=f"pp{t}")
                for t in range(NS_)
            ]

            def gating(t):
                # softmax over the 4 expert rows (no max-sub: |logit| < ~8),
                # partition reductions/broadcasts done with tiny PE matmuls
                tsl = slice(t * NSLAB, (t + 1) * NSLAB)
                pp = pps[t]
                e_sb = small.tile([E, NSLAB], F32, tag="e", name="e_sb")
                nc.scalar.activation(
                    e_sb[:], pp[ER : ER + E, :], mybir.ActivationFunctionType.Exp
                )
                s_ps = psum_proj.tile([1, NSLAB], F32, tag="gat", name="s_ps")
                nc.tensor.matmul(s_ps[:], ones_e1[:], e_sb[:])  # sum_e exp
                r_sb = small.tile([1, NSLAB], F32, tag="r", name="r_sb")
                nc.vector.reciprocal(r_sb[:], s_ps[:])
                r4_ps = psum_proj.tile([E, NSLAB], F32, tag="gat", name="r4_ps")
                nc.tensor.matmul(r4_ps[:], ones_1e[:], r_sb[:])  # bcast to 4 rows
                g4_sb = small.tile([E, NSLAB], F32, tag="g4", name="g4_sb")
                nc.vector.tensor_mul(g4_sb[:], e_sb[:], r4_ps[:])
                # (SCALE * gate)[er, t] via 0/1*SCALE selection matmul
                g32_ps = psum_proj.tile([ER, NSLAB], F32, tag="gat", name="g32_ps")
                nc.tensor.matmul(g32_ps[:], sel_sb[:], g4_sb[:])
                # walrus: tensor_tensor may read at most one operand from PSUM
                g32_sb = small.tile([ER, NSLAB], F32, tag="g32s", name="g32_sb")
                nc.vector.tensor_copy(g32_sb[:], g32_ps[:])
                nc.vector.tensor_mul(wp_sb[0:ER, tsl], pp[0:ER, :], g32_sb[:])

            # ---------- phase 2: base matmul + lora + bias ----------
            def base_kloop(ot):
                if ot not in w_tiles:
                    load_w(ot)
                pots = [
                    psum_base.tile([P, NSLAB], F32, tag=f"po{t}", name=f"po{t}")
                    for t in range(NS_)
                ]
                for k in range(KT):
                    for t in range(NS_):
                        nc.tensor.matmul(
                            pots[t][:],
                            w_tiles[ot][:, k, :],
                            x_sb[:, k, t * NSLAB : (t + 1) * NSLAB],
                            start=(k == 0),
                            stop=False,
                        )
                return pots

            def base_tail(ot, pots):
                # per slab: finish the accumulation (lora+bias row), copy the
                # PSUM bank out, and DMA that half immediately — keeps the
                # end-of-kernel drain to half an o-tile instead of a full one
                osl = slice(ot * P, (ot + 1) * P)
                o_sb = opool.tile([P, TPC], F32, tag="o", name="o_sb")
                for t in range(NS_):
                    tsl = slice(t * NSLAB, (t + 1) * NSLAB)
                    nc.tensor.matmul(
                        pots[t][:],
                        bt_sb[:, osl],
                        wp_sb[:, tsl],
                        start=False,
                        stop=True,
                    )
                    nc.vector.tensor_copy(o_sb[:, tsl], pots[t][:])
                    nc.sync.dma_start(out=od[osl, tsl], in_=o_sb[:, tsl])
                del w_tiles[ot]

            for rep in range(reps):
                if rep == 0:
                    # k-interleaved startup: the proj/router and o-tile-0
                    # matmuls share each x k-tile, so the PE tracks the x DMA
                    # stream instead of running dry; o-tile 1's k-loop follows
                    # un-interleaved to cover the gating chain's ACT/DVE
                    # latency before the o-tile-0/1 lora tails need wp_sb.
                    pots0 = [
                        psum_base.tile([P, NSLAB], F32, tag=f"po{t}", name=f"po{t}")
                        for t in range(NS_)
                    ]
                    for k in range(KT):
                        for t in range(NS_):
                            nc.tensor.matmul(
                                pps[t][:],
                                art_sb[:, k, :],
                                x_sb[:, k, t * NSLAB : (t + 1) * NSLAB],
                                start=(k == 0),
                                stop=(k == KT - 1),
                            )
                        for t in range(NS_):
                            nc.tensor.matmul(
                                pots0[t][:],
                                w_tiles[0][:, k, :],
                                x_sb[:, k, t * NSLAB : (t + 1) * NSLAB],
                                start=(k == 0),
                                stop=False,
                            )
                    pots1 = base_kloop(1)
                    for t in range(NS_):
                        gating(t)
                    base_tail(0, pots0)
                    base_tail(1, pots1)
                    start_ot = 2
                else:
                    start_ot = 0
                for ot in range(start_ot, OT):
                    pots = base_kloop(ot)
                    base_tail(ot, pots)

    nc.compile()
    if DEDUPE_LDW:
        _dedupe_ldweights(nc)
    return nc


def get_nc():
    global _NC
    if _NC is None:
        _NC = build_nc()
    return _NC


def _prep_shared(W, b, A, Bm, Wr):
    # W (OUT, IN) -> wd[ot, p, k, o] = W[ot*128+o, k*128+p], fp16, contiguous
    wd = np.ascontiguousarray(
        W.reshape(OT, P, KT, P).transpose(0, 3, 2, 1).astype(np.float16)
    )
    # [A (E,R,IN) flattened; Wr (E,IN)] -> art[p, k, j] = AR[j, k*128+p]
    ar = np.concatenate([A.reshape(ER, IN), Wr], axis=0)  # (36, IN)
    artd = np.ascontiguousarray(
        ar.T.reshape(KT, P, ER + E).transpose(1, 0, 2).astype(np.float16)
    )
    # Bcat rows er = Bm[e,:,r]; row 32 = bias
    bt = np.concatenate([Bm.transpose(0, 2, 1).reshape(ER, OUT), b[None, :]], axis=0)
    btd = np.ascontiguousarray(bt.astype(np.float16))
    sel = np.zeros((E, ER), np.float32)
    for e in range(E):
        sel[e, e * R : (e + 1) * R] = SCALE
    return wd, artd, btd, sel


def _prep_x_shard(xt, c):
    xs = xt[c * TPC : (c + 1) * TPC]  # (TPC, IN)
    return np.ascontiguousarray(
        xs.T.reshape(KT, P, TPC).transpose(1, 0, 2).astype(np.float16)
    )


def make_in_maps(x, W, b, A, Bm, Wr):
    xt = np.asarray(x, np.float32).reshape(TOK, IN)
    wd, artd, btd, sel = _prep_shared(
        np.asarray(W, np.float32),
        np.asarray(b, np.float32),
        np.asarray(A, np.float32),
        np.asarray(Bm, np.float32),
        np.asarray(Wr, np.float32),
    )
    return [
        {
            "xd": _prep_x_shard(xt, c),
            "wd": wd,
            "artd": artd,
            "btd": btd,
            "seld": sel,
        }
        for c in range(N_CORES)
    ]


def gather_out(results):
    # per-core od is (OUT, TPC); tokens are sharded contiguously
    return np.concatenate([r["od"].T for r in results], axis=0).reshape(B, S, OUT)


def kernel(x, W, b, A, Bm, Wr, _trace=False):
    nc = get_nc()
    in_maps = make_in_maps(x, W, b, A, Bm, Wr)
    res = bass_utils.run_bass_kernel_spmd(
        nc, in_maps, core_ids=list(range(N_CORES)), trace=_trace
    )
    out = gather_out(res.results)
    if _trace:
        return out, res
    return out

